# revision 47
# baseline (speedup 1.0000x reference)
"""Distributed Trainium2 kernel for nn_Attention_61332132987140.

Gated multi-head attention block: RMSNorm -> QKV proj -> RoPE -> softmax
attention -> sigmoid head gating -> output projection.

Sharding: 8 cores = 2 batch groups x 4-head groups (tensor parallel on
heads within a batch). Each core computes attention for its batch's full
sequence over its 4 heads, the partial output projection over its 256
columns of w_out, then a ReduceScatter over its 4-core batch group sums
the partials and leaves each core with a disjoint 128-token slice per
512-token quarter. The host reassembles the full (2, 2048, 1024) output.

Device compute dtype: bf16 operands into the PE array with fp32 PSUM
accumulation; softmax/normalization math in fp32 on ACT/DVE.
"""
import os
import sys

sys.path.insert(0, "/opt/trn_rl_repo")

import numpy as np
import ml_dtypes

import concourse.bass as bass
import concourse.mybir as mybir
import concourse.tile as tile
from concourse import bacc
from concourse.bass_utils import run_bass_kernel_spmd

F32 = mybir.dt.float32
BF16 = mybir.dt.bfloat16
AF = mybir.ActivationFunctionType
ALU = mybir.AluOpType

B, N, DIM = 2, 2048, 1024
HEADS, DH = 16, 64
HL = 4  # local heads per core
P = 128
TT = N // P  # 16 token tiles
KD = DIM // P  # 8 contraction tiles
NQ = 4  # quarters (512-token i-chunks)
QT = N // NQ
CORES = 8
REPLICA_GROUPS = [[0, 1, 2, 3], [4, 5, 6, 7]]

_nc_cache = None
_last_result = None


def _build():
    nc = bacc.Bacc("TRN2", target_bir_lowering=False, debug=False, num_devices=CORES)

    x_ext = nc.declare_dram_parameter("x", [N, DIM], F32, isOutput=False)
    wqkv_ext = nc.declare_dram_parameter("wqkv", [DIM, 3 * HL * DH], BF16, isOutput=False)
    wg_ext = nc.declare_dram_parameter("wg", [DIM, HL], BF16, isOutput=False)
    bgn_ext = nc.declare_dram_parameter("bgn", [HL, 1], F32, isOutput=False)
    wout_ext = nc.declare_dram_parameter("wout", [HL * DH, DIM], BF16, isOutput=False)
    cosr_ext = nc.declare_dram_parameter("cosr", [N, 512], BF16, isOutput=False)
    sinr_ext = nc.declare_dram_parameter("sinr", [N, 512], BF16, isOutput=False)
    out_ext = nc.declare_dram_parameter("out", [NQ, P, DIM], F32, isOutput=True)
    dbg = {}
    if os.environ.get("KDEBUG"):
        dbg["xnT"] = nc.declare_dram_parameter("dbg_xnT", [P, 4, KD, P], BF16,
                                               isOutput=True)
        dbg["qkt"] = nc.declare_dram_parameter("dbg_qkt", [P, TT, 4, P], BF16,
                                               isOutput=True)
        dbg["v"] = nc.declare_dram_parameter("dbg_v", [P, TT, HL * DH], BF16,
                                             isOutput=True)
        dbg["gates"] = nc.declare_dram_parameter("dbg_gates", [P, 512], F32,
                                                 isOutput=True)
        dbg["pt"] = nc.declare_dram_parameter("dbg_pt", [P, 2, 512], BF16,
                                              isOutput=True)
        dbg["sums"] = nc.declare_dram_parameter("dbg_sums", [P, 512], F32,
                                                isOutput=True)
        dbg["av"] = nc.declare_dram_parameter("dbg_av", [P, 512], F32,
                                              isOutput=True)
        dbg["oT"] = nc.declare_dram_parameter("dbg_oT", [P, 2, 512], BF16,
                                              isOutput=True)
        dbg["y"] = nc.declare_dram_parameter("dbg_y", [QT, DIM], F32,
                                             isOutput=True)

    with tile.TileContext(nc) as tc:
        with (
            tc.tile_pool(name="wpool", bufs=1) as wpool,
            tc.tile_pool(name="persist", bufs=1) as persist,
            tc.tile_pool(name="xstream", bufs=4) as xstream,
            tc.tile_pool(name="stream", bufs=3) as stream,
            tc.tile_pool(name="xntp", bufs=2) as xntp,
            tc.tile_pool(name="ptp", bufs=26) as ptp,
            tc.tile_pool(name="tail", bufs=2) as tailp,
            tc.tile_pool(name="ps_qa1", bufs=1, space="PSUM") as ps_qa1,
            tc.tile_pool(name="ps_qa2", bufs=1, space="PSUM") as ps_qa2,
            tc.tile_pool(name="ps_s", bufs=2, space="PSUM") as ps_s,
            tc.tile_pool(name="ps_cps", bufs=1, space="PSUM") as ps_cps,
            tc.tile_pool(name="ps_attn", bufs=1, space="PSUM") as ps_attn,
            tc.tile_pool(name="ps_ygs", bufs=1, space="PSUM") as ps_ygs,
            tc.tile_pool(name="dram", bufs=1, space="DRAM") as dramp,
        ):
            # ---- constants / weights ----
            wqkv_sb = wpool.tile([P, KD, 768], BF16)
            nc.scalar.dma_start(
                wqkv_sb[:], wqkv_ext.rearrange("(k p) f -> p k f", p=P)
            )
            wg_sb = wpool.tile([P, KD, HL], BF16)
            nc.scalar.dma_start(wg_sb[:], wg_ext.rearrange("(k p) f -> p k f", p=P))
            wout_sb = wpool.tile([P, 2, DIM], BF16)
            nc.scalar.dma_start(
                wout_sb[:], wout_ext.rearrange("(k p) f -> p k f", p=P)
            )
            bgn_sb = wpool.tile([HL, 1], F32)
            nc.scalar.dma_start(bgn_sb[:], bgn_ext[:])
            zb = wpool.tile([P, 1], F32)
            nc.gpsimd.memset(zb[:], 0.0)
            lb32 = wpool.tile([P, 1], F32)
            nc.gpsimd.memset(lb32[:], float(np.log(32.0)))
            ones_sb = wpool.tile([P, 1], BF16)
            nc.gpsimd.memset(ones_sb[:], 1.0)
            ones_row = wpool.tile([1, 64], BF16)
            nc.gpsimd.memset(ones_row[:], 1.0)

            # ---- persistent activations ----
            # QKT_sb[p, tok_tile, blk, t]: blk 0/1 = q head-pairs; 2/3 = k.
            # blk-last layout keeps each xbar-transpose destination dense
            QKT_sb = persist.tile([P, TT, 4, P], BF16)
            # v_sb[j_in_tile, jt, h*64+d]
            v_sb = persist.tile([P, TT, HL * DH], BF16)
            # gates for head h live at partition 32*h; other rows are garbage
            gates_sb = persist.tile([P, N], F32)

            def emit_se(ci, jt, dbg_tap=False):
                """scores + exp for one (quarter, j-tile); returns pt tiles"""
                pts = []
                for hp in range(2):
                    s_ps = ps_s.tile([P, 2, 512], F32, name="s_ps", tag="s")
                    for e in range(2):
                        nc.tensor.matmul(
                            s_ps[:, e, :],
                            QKT_sb[e * 64:(e + 1) * 64, jt, 2 + hp, :],
                            QKT_sb[e * 64:(e + 1) * 64, 4 * ci:4 * ci + 4,
                                   hp, :],
                            start=True, stop=True,
                        )
                    pt = ptp.tile([P, 2, 512], BF16, name="pt")
                    nc.scalar.activation(pt[:], s_ps[:], AF.Exp, scale=0.125,
                                         bias=zb[:])
                    pts.append(pt)
                    if dbg_tap and hp == 0:
                        nc.gpsimd.dma_start(dbg["pt"][:], pt[:])
                return pts

            def emit_avs(jt, pts, av01, av23, sums):
                """AV accumulation + softmax-sum matmuls for one j-tile"""
                for hp in range(2):
                    avt = av01 if hp == 0 else av23
                    # adjacent issue of col-disjoint AV matmuls -> concurrent.
                    # Concurrent accumulation groups in one bank are fine:
                    # partition ranges disjoint; has_written is per row
                    for e in range(2):
                        h = 2 * hp + e
                        nc.tensor.matmul(
                            avt[e * 64:(e + 1) * 64, :],
                            v_sb[:, jt, h * DH:(h + 1) * DH],
                            pts[hp][:, e, :],
                            start=(jt == 0), stop=(jt == TT - 1),
                            skip_group_check=True,
                        )
                # four sum matmuls back-to-back: distinct 32-col groups ->
                # one concurrent 512-cycle slot
                for hp in range(2):
                    for e in range(2):
                        h = 2 * hp + e
                        nc.tensor.matmul(
                            sums[h * 32:h * 32 + 1, :],
                            ones_sb[:, 0:1],
                            pts[hp][:, e, :],
                            start=(jt == 0), stop=(jt == TT - 1),
                            tile_position=(0, h * 32),
                            skip_group_check=True,
                        )

            # =========== Phase A: norm, QKV, RoPE, transposes ===========
            pre_pts = {}
            for ci in range(NQ):
                # xnT[p, tt, kd, t]: kd-last so each transpose dest is dense
                xnT = xntp.tile([P, 4, KD, P], BF16, name="xnT")
                # batch the 4 tiles' Ln/Exp into single ops: the ACT table
                # pass places a LoadActFuncSet at every Ln<->Exp alternation,
                # so grouping keeps it to 2 loads per chunk
                xts = []
                ss4 = stream.tile([P, 4], F32, name="ss4")
                for tt in range(4):
                    tok = ci * 4 + tt
                    x_t = xstream.tile([P, DIM], F32, name="x_t")
                    nc.gpsimd.dma_start(x_t[:], x_ext[tok * P:(tok + 1) * P, :])
                    xts.append(x_t)
                    scr = stream.tile([P, DIM], BF16, name="scr")
                    nc.vector.tensor_tensor(out=scr[:], in0=x_t[:], in1=x_t[:],
                                            op=ALU.mult)
                    nc.vector.reduce_sum(ss4[:, tt:tt + 1], scr[:],
                                         axis=mybir.AxisListType.X)
                ln4 = stream.tile([P, 4], F32, name="ln4")
                nc.scalar.activation(ln4[:], ss4[:], AF.Ln, bias=zb[:])
                sc4 = stream.tile([P, 4], F32, name="sc4")
                nc.scalar.activation(sc4[:], ln4[:], AF.Exp, scale=-0.5,
                                     bias=lb32[:])
                for tt in range(4):
                    xn_t = stream.tile([P, DIM], BF16, name="xn_t")
                    nc.scalar.activation(xn_t[:], xts[tt][:], AF.Copy,
                                         scale=sc4[:, tt:tt + 1])
                    nc.sync.dma_start_transpose(xnT[:, tt, :, :], xn_t[:])

                for tt in range(4):
                    tok = ci * 4 + tt
                    qk_ps = ps_qa1.tile([P, 512], F32, name="qk_ps", tag="qa1")
                    v_ps = ps_qa2.tile([P, 256], F32, name="v_ps", tag="qa2")
                    for kd in range(KD):
                        lhsT = xnT[:, tt, kd, :]
                        nc.tensor.matmul(qk_ps[:], lhsT,
                                         wqkv_sb[:, kd, 0:512],
                                         start=(kd == 0), stop=(kd == KD - 1))
                        nc.tensor.matmul(v_ps[:], lhsT,
                                         wqkv_sb[:, kd, 512:768],
                                         start=(kd == 0), stop=(kd == KD - 1))
                    # RoPE on q|k (psum cols 0:512), even/odd feature halves
                    cos_t = stream.tile([P, 512], BF16, name="cos_t")
                    nc.gpsimd.dma_start(cos_t[:], cosr_ext[tok * P:(tok + 1) * P, :])
                    sin_t = stream.tile([P, 512], BF16, name="sin_t")
                    nc.gpsimd.dma_start(sin_t[:], sinr_ext[tok * P:(tok + 1) * P, :])
                    qkv = qk_ps[:].rearrange("p (b c) -> p b c", b=8)
                    qE, qO = qkv[:, :, 0:32], qkv[:, :, 32:64]
                    cE = cos_t[:, 0:256].rearrange("p (b c) -> p b c", b=8)
                    cO = cos_t[:, 256:512].rearrange("p (b c) -> p b c", b=8)
                    sE = sin_t[:, 0:256].rearrange("p (b c) -> p b c", b=8)
                    sO = sin_t[:, 256:512].rearrange("p (b c) -> p b c", b=8)
                    t1 = stream.tile([P, 256], F32, name="t1")
                    t2 = stream.tile([P, 256], F32, name="t2")
                    t1v = t1[:].rearrange("p (b c) -> p b c", b=8)
                    t2v = t2[:].rearrange("p (b c) -> p b c", b=8)
                    qk_sb = stream.tile([P, 512], BF16, name="qk_sb")
                    qkv_out = qk_sb[:].rearrange("p (b c) -> p b c", b=8)
                    outE, outO = qkv_out[:, :, 0:32], qkv_out[:, :, 32:64]
                    nc.vector.tensor_tensor(out=t1v, in0=qE, in1=cE, op=ALU.mult)
                    nc.vector.tensor_tensor(out=t2v, in0=qO, in1=sE, op=ALU.mult)
                    nc.vector.tensor_tensor(out=outE, in0=t1v, in1=t2v,
                                            op=ALU.subtract)
                    t3 = stream.tile([P, 256], F32, name="t1")
                    t4 = stream.tile([P, 256], F32, name="t2")
                    t3v = t3[:].rearrange("p (b c) -> p b c", b=8)
                    t4v = t4[:].rearrange("p (b c) -> p b c", b=8)
                    nc.vector.tensor_tensor(out=t3v, in0=qO, in1=cO, op=ALU.mult)
                    nc.vector.tensor_tensor(out=t4v, in0=qE, in1=sO, op=ALU.mult)
                    nc.vector.tensor_tensor(out=outO, in0=t3v, in1=t4v, op=ALU.add)
                    # v: psum cols 512:768 -> v_sb
                    nc.vector.tensor_copy(v_sb[:, tok, :], v_ps[:])
                    # transpose rotated q|k into QKT
                    nc.sync.dma_start_transpose(QKT_sb[:, tok, :, :], qk_sb[:])

                # gates for this chunk: sigmoid(xn @ wg.T + b) via exp, then
                # scatter head h to partition 32*h of gates_sb (DVE operand
                # bases must be 32-aligned, DMA moves partitions freely)
                gates_ps = ps_ygs.tile([HL, 512], F32, name="gates_ps", tag="ygs")
                for kd in range(KD):
                    nc.tensor.matmul(gates_ps[:], wg_sb[:, kd, :],
                                     xnT[:, :, kd, :],
                                     start=(kd == 0), stop=(kd == KD - 1))
                ge = stream.tile([HL, 512], F32, name="ge")
                nc.scalar.activation(ge[:], gates_ps[:], AF.Exp, scale=-1.0,
                                     bias=bgn_sb[:])
                gp = stream.tile([HL, 512], F32, name="gp")
                nc.vector.tensor_scalar_add(gp[:], ge[:], 1.0)
                grec = stream.tile([HL, 512], F32, name="grec")
                nc.vector.reciprocal(grec[:], gp[:])
                nc.gpsimd.dma_start(
                    gates_sb[:, ci * 512:(ci + 1) * 512]
                    .rearrange("(a b) c -> a b c", b=32)[:, 0, :],
                    grec[:],
                )
                if dbg and ci == 0:
                    nc.gpsimd.dma_start(dbg["xnT"][:], xnT[:])

            # =========== Phase B: attention + out proj + RS ===========
            ydram = []
            rsout = []
            for ci in range(NQ):
                ydram.append(dramp.tile([QT, DIM], F32, name=f"ydram{ci}"))
                rsout.append(dramp.tile([P, DIM], F32, name=f"rsout{ci}"))

            if dbg:
                nc.gpsimd.dma_start(dbg["qkt"][:], QKT_sb[:])
                nc.gpsimd.dma_start(dbg["v"][:], v_sb[:])
                nc.gpsimd.dma_start(dbg["gates"][:], gates_sb[:, 0:512])

            nq_run = int(os.environ.get("KQUARTERS", NQ))
            for ci in range(nq_run):
                av01 = ps_qa1.tile([P, 512], F32, name="av01", tag="qa1")
                av23 = ps_qa2.tile([P, 512], F32, name="av23", tag="qa2")
                sums = ps_ygs.tile([97, 512], F32, name="sums", tag="ygs")
                islc = slice(ci * 512, (ci + 1) * 512)
                for jt in range(TT):
                    if (ci, jt) in pre_pts:
                        pts = pre_pts.pop((ci, jt))
                    else:
                        pts = emit_se(ci, jt)
                    emit_avs(jt, pts, av01, av23, sums)

                # prefetch the next quarter's first scores+exps so the ACT
                # queue has no gap across the quarter boundary
                if ci + 1 < nq_run:
                    for jt in range(4):
                        pre_pts[(ci + 1, jt)] = emit_se(ci + 1, jt)

                if dbg and ci == 0:
                    smd = tailp.tile([P, 512], F32, name="smd")
                    nc.vector.tensor_copy(smd[0:97, :], sums[0:97, :])
                    nc.gpsimd.dma_start(dbg["sums"][:], smd[:])
                    avd = tailp.tile([P, 512], F32, name="avd")
                    nc.vector.tensor_copy(avd[:], av01[:])
                    nc.gpsimd.dma_start(dbg["av"][:], avd[:])

                # normalize + gate -> outflatT. The per-query scale
                # c = gate/softmax_sum is broadcast across the 64 head dims
                # with a K=1 ones outer-product on the PE.
                oT = tailp.tile([P, 2, 512], BF16, name="oT")
                for hp in range(2):
                    c_ps = ps_cps.tile([P, 512], F32, name="c_ps", tag="cps")
                    for e in range(2):
                        h = 2 * hp + e
                        sr = tailp.tile([1, 512], F32, name="sr")
                        nc.vector.tensor_copy(sr[:], sums[h * 32:h * 32 + 1, :])
                        gr = tailp.tile([1, 512], F32, name="gr")
                        nc.vector.tensor_copy(gr[:], gates_sb[h * 32:h * 32 + 1, islc])
                        rc = tailp.tile([1, 512], F32, name="rc")
                        nc.vector.reciprocal(rc[:], sr[:])
                        cr = tailp.tile([1, 512], BF16, name="cr")
                        nc.vector.tensor_tensor(out=cr[:], in0=rc[:], in1=gr[:],
                                                op=ALU.mult)
                        nc.tensor.matmul(c_ps[e * 64:(e + 1) * 64, :],
                                         ones_row[:, :], cr[:],
                                         start=True, stop=True)
                    c_sb = tailp.tile([P, 512], F32, name="c_sb")
                    nc.vector.tensor_copy(c_sb[:], c_ps[:])
                    avt = av01 if hp == 0 else av23
                    nc.vector.tensor_tensor(out=oT[:, hp, :], in0=avt[:],
                                            in1=c_sb[:], op=ALU.mult)

                # output projection (partial over local heads)
                for tt in range(4):
                    for oc in range(2):
                        y_ps = ps_ygs.tile([P, 512], F32, name="y_ps", tag="ygs")
                        for kt in range(2):
                            nc.tensor.matmul(
                                y_ps[:],
                                oT[:, kt, tt * P:(tt + 1) * P],
                                wout_sb[:, kt, oc * 512:(oc + 1) * 512],
                                start=(kt == 0), stop=(kt == 1),
                            )
                        y_sb = tailp.tile([P, 512], F32, name="y_sb")
                        nc.vector.tensor_copy(y_sb[:], y_ps[:])
                        nc.sync.dma_start(
                            ydram[ci][tt * P:(tt + 1) * P,
                                      oc * 512:(oc + 1) * 512],
                            y_sb[:],
                        )

                if dbg and ci == 0:
                    nc.gpsimd.dma_start(dbg["oT"][:], oT[:])
                    nc.gpsimd.dma_start(dbg["y"][:], ydram[ci][:])

                if os.environ.get("KNOCOLL"):
                    nc.gpsimd.dma_start(out_ext[ci, :, :], ydram[ci][0:P, :])
                else:
                    nc.gpsimd.collective_compute(
                        "ReduceScatter", ALU.add,
                        replica_groups=REPLICA_GROUPS,
                        ins=[ydram[ci][:].opt()],
                        outs=[rsout[ci][:].opt()],
                    )
                    nc.sync.dma_start(out_ext[ci, :, :], rsout[ci][:])

    nc.compile()
    return nc


def _get_nc():
    global _nc_cache
    if _nc_cache is None:
        _nc_cache = _build()
    return _nc_cache


_PERM_EO = np.concatenate([np.arange(0, DH, 2), np.arange(1, DH, 2)])


def _shard(core, x, rotary_cos, rotary_sin, gamma, w_qkv, w_gates, b_gates, w_out):
    g, r = core // 4, core % 4
    heads = np.arange(4 * r, 4 * r + 4)
    wq = w_qkv[0 * DIM:1 * DIM] * gamma[None, :]
    wk = w_qkv[1 * DIM:2 * DIM] * gamma[None, :]
    wv = w_qkv[2 * DIM:3 * DIM]

    def qk_rows(w):
        # rows for local heads with even/odd permutation within each head
        idx = (heads[:, None] * DH + _PERM_EO[None, :]).reshape(-1)
        return w[idx]

    v_rows = wv[(heads[:, None] * DH + np.arange(DH)[None, :]).reshape(-1)]
    wqkv_t = np.concatenate([qk_rows(wq), qk_rows(wk), v_rows], axis=0).T
    wg_t = (w_gates[heads] * gamma[None, :]).T
    wout_t = w_out[:, heads[0] * DH:heads[0] * DH + HL * DH].T

    cos = rotary_cos[0, 0]  # (N, DH)
    sin = rotary_sin[0, 0]
    cosr = np.concatenate([np.tile(cos[:, 0::2], (1, 8)),
                           np.tile(cos[:, 1::2], (1, 8))], axis=1)
    sinr = np.concatenate([np.tile(sin[:, 0::2], (1, 8)),
                           np.tile(sin[:, 1::2], (1, 8))], axis=1)

    bf = ml_dtypes.bfloat16
    return {
        "x": np.ascontiguousarray(x[g], np.float32),
        "wqkv": np.ascontiguousarray(wqkv_t).astype(bf),
        "wg": np.ascontiguousarray(wg_t).astype(bf),
        "bgn": np.ascontiguousarray(-b_gates[heads].reshape(HL, 1), np.float32),
        "wout": np.ascontiguousarray(wout_t).astype(bf),
        "cosr": np.ascontiguousarray(cosr).astype(bf),
        "sinr": np.ascontiguousarray(sinr).astype(bf),
    }


def kernel(x, rotary_cos, rotary_sin, gamma, w_qkv, w_gates, b_gates, w_out):
    global _last_result
    args = [np.asarray(a, np.float32) for a in
            (x, rotary_cos, rotary_sin, gamma, w_qkv, w_gates, b_gates, w_out)]
    nc = _get_nc()
    in_maps = [_shard(c, *args) for c in range(CORES)]
    try:
        res = run_bass_kernel_spmd(
            nc, in_maps, core_ids=list(range(CORES)),
            trace=bool(os.environ.get("KTRACE")),
        )
    except ModuleNotFoundError:
        # profiler hook unavailable in this environment - run without trace
        res = run_bass_kernel_spmd(nc, in_maps, core_ids=list(range(CORES)))
    _last_result = res
    full = np.zeros((B, N, DIM), np.float32)
    for c in range(CORES):
        g, r = c // 4, c % 4
        o = np.asarray(res.results[c]["out"]).reshape(NQ, P, DIM)
        for q in range(NQ):
            full[g, q * 512 + r * P: q * 512 + (r + 1) * P, :] = o[q]
    return full
